# revision 29
# baseline (speedup 1.0000x reference)
"""Trainium2 Bass kernel for nn_CAAN_78323023610440.

Reference computation (per batch b):
    q = x @ Wq.T + bq;  k = x @ Wk.T + bk;  v = x @ Wv.T + bv
    beta = softmax(q @ k.T / sqrt(D), axis=-1)
    final = (beta @ v) @ Ww.T + bw            # [B, N]

Algebraic restructuring (exact, modulo fp reassociation):
  *  q.k = x A x^T + r[n] + c[m] + const, with A = Wq^T Wk,
     r[n] = x[n].(Wq^T bk) (row-constant -> drops out of softmax),
     c[m] = x[m].(Wk^T bq) (key-side constant, kept). c is LINEAR in
     x[m], so it folds into the TT operand: c[m] = sum_e g[e] x[m,e]
     with g = Wk^T bq -> add g to every column of TT (see below).
  *  (beta @ v) @ Ww^T = beta @ (x @ (Wv^T Ww^T) + bv.Ww)
     -> the whole V projection collapses into a per-key scalar wv[m].
  *  final[n] = sum_m exp(s[n,m]) wv[m] / sum_m exp(s[n,m]) + bw
     (softmax max-subtraction skipped: logits are O(1) here, exp is safe
      in fp32 — both sums are formed unnormalized and divided at the end).

Sharding: 8 cores = 4 batches x 2 query-halves. Each core computes, for
its 1024 queries n and all 2048 keys m of its batch (keys ordered
local-half-first so all 8 cores run an identical SPMD program):
    TT[e, n]  = sum_f A[f, e] xT[f, n] + g[e]     (phase 1 + DVE add)
    S[n, m]   = sum_e TT[e, n] xT[e, m]           (phase 2, QUERIES on
                                                   partitions, keys free)
    pt        = Exp(S/32/32)                      (ScalarE; accum_out
                                                   gives den = sum_m pt)
    num[n]    = sum_m pt[n, m] wv[m]              (DVE tensor_tensor_reduce
                                                   vs wv replicated row)
Host divides num/den and adds bw.

The orientation (queries on PSUM partitions) makes both softmax sums
FREE-dim reductions, so no PE matmuls are spent on them — the old
key-partition layout burned ~20% of PE time on [128,2]-stationary
reduction matmuls at 0.8% PE efficiency.

All heavy matmuls run in fp8e4 (e4m3) with MatmulPerfMode.DoubleRow:
each instruction contracts TWO 128-deep tiles (stationary [128,2,128],
moving [128,2,512]) at ~1 column/cycle — the 157 TF/s fp8 peak. A is
pre-scaled by 32 on the host so its entries (std ~0.01) sit in e4m3's
normal range; the exp scale compensates (1/1024). Measured numerics:
~1.4e-2 max rel err vs the 2e-2 gate (dominated by fp8 quantization of
x and TT; pt/wv stay bf16, accumulation fp32).

Inputs are host-prearranged into partition-contiguous SBUF images so
the whole input set streams in 8 large DMAs (the per-slice DMA fan-out
of the v1 kernel serialized ~35 descriptor pushes at ~600ns each on the
Sync engine and starved phase 1).
"""

import numpy as np
from contextlib import ExitStack

import ml_dtypes

import concourse.tile as tile
from concourse import bacc, mybir
from concourse.bass_utils import run_bass_kernel_spmd

B = 4
N = 2048
D = 1024
P = 128
ET = D // P          # 8 contraction tiles over D
NQ = N // 2          # 1024 local queries per core
NT = NQ // P         # 8 query tiles
CHUNK = 512          # PSUM bank limit (512 fp32 outputs)
KC = N // CHUNK      # 4 key chunks
NCH = NQ // CHUNK    # 2 query chunks (phase 1 moving)
NP = ET // 2         # 4 DoubleRow contraction pairs
E0T = 3              # e-tiles in phase-1 block 0 (A split a0/a1)
SCALE = 0.03125      # 1/sqrt(D), exact
A_SCALE = 32.0       # host multiplies A,g by this; exp scale divides
WARMUP_MM = 8        # dummy matmuls to lift the PE HAM clock-gate early
F32 = mybir.dt.float32
BF16 = mybir.dt.bfloat16
FP8 = mybir.dt.float8e4
EXP = mybir.ActivationFunctionType.Exp
IDN = mybir.ActivationFunctionType.Identity
DR = mybir.MatmulPerfMode.DoubleRow
ADD = mybir.AluOpType.add
MULT = mybir.AluOpType.mult
AXX = mybir.AxisListType.X

_CACHE = {}


def _build():
    nc = bacc.Bacc(
        "TRN2",
        target_bir_lowering=False,
        debug=False,
        enable_asserts=False,
        num_devices=8,
    )
    # Host-prearranged, partition-contiguous inputs (see _prep):
    #   xq [p, f, n] = x[query n, f*128+p]     (local query half)
    #   xk [p, f, m] = x[key m, f*128+p]       (other half, m >= NQ)
    #   a0 [p, f, e] = 32*A[f*128+p, e]        e in [0, 384)
    #   a1 [p, f, e] = 32*A[f*128+p, 384+e]    e in [0, 640)
    #   gt [p, et]   = 32*g[et*128+p]
    #   wv [p, m]    = wv[m]  (replicated rows, for the DVE reduce)
    # xq/xk are separate tensors so phase 1's critical DMA bytes are
    # only a0+xq (1.4MB) — the key half streams during phase-1 compute.
    xq_d = nc.dram_tensor("xq", [P, ET, NQ], FP8, kind="ExternalInput")
    xk_d = nc.dram_tensor("xk", [P, ET, NQ], FP8, kind="ExternalInput")
    a0_d = nc.dram_tensor("a0", [P, ET, E0T * P], FP8, kind="ExternalInput")
    a1_d = nc.dram_tensor("a1", [P, ET, (ET - E0T) * P], FP8,
                          kind="ExternalInput")
    gt_d = nc.dram_tensor("gt", [P, ET], F32, kind="ExternalInput")
    wv_d = nc.dram_tensor("wv", [P, N], BF16, kind="ExternalInput")
    out_d = nc.dram_tensor("out", [P, NT, 2], F32, kind="ExternalOutput")

    with tile.TileContext(nc) as tc, ExitStack() as ctx:
        const = ctx.enter_context(tc.tile_pool(name="const", bufs=1))
        ptp = ctx.enter_context(tc.tile_pool(name="pt", bufs=4))
        scrp = ctx.enter_context(tc.tile_pool(name="scr", bufs=2))
        workp = ctx.enter_context(
            tc.tile_pool(name="psum_work", bufs=6, space="PSUM")
        )
        wup = ctx.enter_context(
            tc.tile_pool(name="psum_wu", bufs=1, space="PSUM")
        )

        x_sb = const.tile([P, ET, N], FP8)      # xT, queries 0:NQ / keys all
        a0_sb = const.tile([P, ET, E0T * P], FP8)
        a1_sb = const.tile([P, ET, (ET - E0T) * P], FP8)
        tt_sb = const.tile([P, ET, NQ], FP8)    # TT' = 32(A^T xq^T + g)
        gt_sb = const.tile([P, ET], F32)
        wv_sb = const.tile([P, N], BF16)
        den_sb = const.tile([P, NT, KC], F32)   # per-chunk denominators
        num_sb = const.tile([P, NT, KC], F32)   # per-chunk numerators
        out_sb = const.tile([P, NT, 2], F32)
        wu_sb = const.tile([P, CHUNK], BF16)    # warmup operand (garbage ok)
        wu_sink = const.tile([P, 1], F32)

        # PE warm-up: keep TensorE busy from t~0 so the HAM clock-gate
        # lifts to 8/8 before the real matmuls start (they are DMA-gated),
        # and fill the DMA-paced holes of the first TT block below.
        nc.gpsimd.memset(wu_sb[:], 0.0)
        wu_ps = wup.tile([P, CHUNK], F32)
        wpf = 2
        n_wu = WARMUP_MM + wpf * NP
        wu_iter = iter(range(n_wu))

        def warm(k):
            for _ in range(k):
                w = next(wu_iter, None)
                if w is None:
                    return
                nc.tensor.matmul(
                    wu_ps[:],
                    wu_sb[:, :P],
                    wu_sb[:],
                    start=(w == 0),
                    stop=(w == n_wu - 1),
                )

        warm(WARMUP_MM)

        # Input DMAs: few and large (contiguous per partition). a0 gates
        # EVERY phase-1 matmul (single-DMA dependency) so it goes first,
        # then the query halves (phase-1 critical path), then the key
        # halves + a1 (needed by phase 2 / blocks 1-2), wv last (first
        # numerator reduce is ~20us in).
        nc.sync.dma_start(a0_sb[:], a0_d[:])
        for i in range(NP):
            nc.sync.dma_start(x_sb[:, 2 * i:2 * i + 2, :NQ],
                              xq_d[:, 2 * i:2 * i + 2, :])
            if i == 0:
                nc.sync.dma_start(gt_sb[:], gt_d[:])
        nc.sync.dma_start(a1_sb[:], a1_d[:])
        for i in range(NP):
            nc.sync.dma_start(x_sb[:, 2 * i:2 * i + 2, NQ:],
                              xk_d[:, 2 * i:2 * i + 2, :])
        nc.sync.dma_start(wv_sb[:], wv_d[:])

        def a_slc(ci, e):
            if e < E0T:
                return a0_sb[:, 2 * ci:2 * ci + 2, e * P:(e + 1) * P]
            eo = e - E0T
            return a1_sb[:, 2 * ci:2 * ci + 2, eo * P:(eo + 1) * P]

        # Phase 1: TT[e, n] = sum_f A[f, e-cols]^T . xT[f, n] over the
        # local-query columns. e-blocks of 3 keep 6 PSUM groups open so
        # each arriving (A, x) pair DMA feeds 6 matmuls.
        BLOCKS = [(0, 3), (3, 3), (6, 2)]
        for eb, (e0, blk) in enumerate(BLOCKS):
            pss = [
                [workp.tile([P, CHUNK], F32, name=f"tt_ps_{eb}_{el}_{j}",
                            tag="ps") for j in range(NCH)]
                for el in range(blk)
            ]
            for ci in range(NP):
                for el in range(blk):
                    e = e0 + el
                    for j in range(NCH):
                        nc.tensor.matmul(
                            pss[el][j][:],
                            a_slc(ci, e),
                            x_sb[:, 2 * ci:2 * ci + 2,
                                 j * CHUNK:(j + 1) * CHUNK],
                            start=(ci == 0),
                            stop=(ci == NP - 1),
                            perf_mode=DR,
                        )
                if eb == 0:
                    # absorb the DMA-arrival pacing of the first block
                    warm(wpf)
            for el in range(blk):
                e = e0 + el
                for j in range(NCH):
                    # fused add of the key-side constant g + fp8 cast.
                    # Stays on DVE: counterintuitively, moving these to
                    # the otherwise-idle ScalarE measured SLOWER overall —
                    # the early cast ops keep the DVE queue primed and
                    # its back-to-back pipelining helps it drain the
                    # phase-2 numerator backlog faster.
                    nc.vector.tensor_scalar(
                        tt_sb[:, e, j * CHUNK:(j + 1) * CHUNK],
                        pss[el][j][:],
                        gt_sb[:, e:e + 1],
                        None,
                        ADD,
                    )
            if eb == 0:
                warm(100)  # flush any leftover warmups
                nc.vector.tensor_copy(wu_sink[:], wu_ps[:, :1])

        # Phase 2: per (query-tile nt, key-chunk ch): S accumulates in
        # PSUM, exp with free-dim accumulation (denominator), DVE
        # multiply + chunk reduce (numerator partial). Per-chunk DVE ops
        # pay more fixed overhead than full-width per-nt passes, but the
        # measured end-to-end is FASTER this way — wider passes bunch at
        # tile boundaries and the last tile's work queues behind them.
        # (tensor_tensor_reduce would fuse the two DVE ops, but that
        # opcode hard-faults this runtime's exec unit.)
        for nt in range(NT):
            for ch in range(KC):
                ps = workp.tile([P, CHUNK], F32, name=f"s_ps_{nt}_{ch}",
                                tag="ps")
                for ci in range(NP):
                    nc.tensor.matmul(
                        ps[:],
                        tt_sb[:, 2 * ci:2 * ci + 2, nt * P:(nt + 1) * P],
                        x_sb[:, 2 * ci:2 * ci + 2,
                             ch * CHUNK:(ch + 1) * CHUNK],
                        start=(ci == 0),
                        stop=(ci == NP - 1),
                        perf_mode=DR,
                    )
                pt = ptp.tile([P, CHUNK], BF16, name=f"pt_{nt}_{ch}",
                              tag="pt")
                nc.scalar.activation(
                    pt[:], ps[:], EXP, scale=SCALE / A_SCALE,
                    accum_out=den_sb[:, nt, ch:ch + 1],
                )
                scr = scrp.tile([P, CHUNK], BF16, name=f"scr_{nt}_{ch}",
                                tag="scr")
                nc.vector.tensor_tensor(
                    scr[:], pt[:], wv_sb[:, ch * CHUNK:(ch + 1) * CHUNK],
                    MULT,
                )
                nc.vector.tensor_reduce(
                    num_sb[:, nt, ch:ch + 1], scr[:], AXX, ADD,
                )

        # Phase 3: fold the KC per-chunk partials and ship (num, den).
        nc.vector.tensor_reduce(out_sb[:, :, 0:1], num_sb[:], AXX, ADD)
        nc.vector.tensor_reduce(out_sb[:, :, 1:2], den_sb[:], AXX, ADD)
        nc.sync.dma_start(out_d[:], out_sb[:])

    nc.compile()
    return nc


def _get_nc():
    if "nc" not in _CACHE:
        _CACHE["nc"] = _build()
    return _CACHE["nc"]


def _prep(x, Wq, bq, Wk, bk, Wv, bv, Ww, bw):
    """Host-side sharding + weight folding -> per-core input maps."""
    x = np.asarray(x, dtype=np.float32)
    Wq = np.asarray(Wq, dtype=np.float32)
    bq = np.asarray(bq, dtype=np.float32)
    Wk = np.asarray(Wk, dtype=np.float32)
    bk = np.asarray(bk, dtype=np.float32)
    Wv = np.asarray(Wv, dtype=np.float32)
    bv = np.asarray(bv, dtype=np.float32)
    Ww = np.asarray(Ww, dtype=np.float32)
    f8 = ml_dtypes.float8_e4m3
    bf = ml_dtypes.bfloat16

    # Host-side weight folding (cheap: one 1024^3 sgemm + matvecs).
    A = (Wq.T @ Wk) * A_SCALE                       # [f, e], fp8-range
    g = (Wk.T @ bq) * A_SCALE                       # key-side logit constant
    wv_eff = Wv.T @ Ww[0]                           # collapsed V @ Ww^T
    cvw = float(bv @ Ww[0])
    wv_all = x @ wv_eff + cvw                       # [B, N]

    a_im = np.ascontiguousarray(
        A.reshape(ET, P, D).transpose(1, 0, 2))     # [p, f, e]
    a0 = np.ascontiguousarray(a_im[:, :, :E0T * P]).astype(f8)
    a1 = np.ascontiguousarray(a_im[:, :, E0T * P:]).astype(f8)
    gt = np.ascontiguousarray(g.reshape(ET, P).T)   # [p, et] f32

    in_maps = []
    for core in range(8):
        b, h = divmod(core, 2)
        lo = np.arange(h * NQ, (h + 1) * NQ)
        hi = np.arange((1 - h) * NQ, (2 - h) * NQ)
        order = np.concatenate([lo, hi])            # keys: local half first
        xqim = np.ascontiguousarray(
            x[b][lo].T.reshape(ET, P, NQ).transpose(1, 0, 2)
        ).astype(f8)                                # [p, f, n]
        xkim = np.ascontiguousarray(
            x[b][hi].T.reshape(ET, P, NQ).transpose(1, 0, 2)
        ).astype(f8)                                # [p, f, m-NQ]
        wvr = np.ascontiguousarray(
            np.broadcast_to(wv_all[b][order][None, :].astype(bf), (P, N))
        )
        in_maps.append({"xq": xqim, "xk": xkim, "a0": a0, "a1": a1,
                        "gt": gt, "wv": wvr})
    return in_maps


def _gather(res, bw):
    bw = np.asarray(bw, dtype=np.float32)
    final = np.empty((B, N), dtype=np.float32)
    for core in range(8):
        b, h = divmod(core, 2)
        o = res.results[core]["out"]                # [p, nt, 2]
        num = o[:, :, 0].T.reshape(NQ)              # n = nt*128 + p
        den = o[:, :, 1].T.reshape(NQ)
        final[b, h * NQ:(h + 1) * NQ] = num / den + bw[0]
    return final


def kernel(x, Wq, bq, Wk, bk, Wv, bv, Ww, bw):
    nc = _get_nc()
    in_maps = _prep(x, Wq, bq, Wk, bk, Wv, bv, Ww, bw)
    res = run_bass_kernel_spmd(nc, in_maps, core_ids=list(range(8)))
    return _gather(res, bw)


def run_profiled(inputs, trace_cores=(0,)):
    """Run once with NTFF profiling; returns BassKernelResults."""
    nc = _get_nc()
    in_maps = _prep(**inputs)
    res = run_bass_kernel_spmd(
        nc, in_maps, core_ids=list(range(8)), trace=True,
        trace_cores=list(trace_cores),
    )
    return res


# revision 32
# speedup vs baseline: 1.0120x; 1.0120x over previous
"""Trainium2 Bass kernel for nn_CAAN_78323023610440.

Reference computation (per batch b):
    q = x @ Wq.T + bq;  k = x @ Wk.T + bk;  v = x @ Wv.T + bv
    beta = softmax(q @ k.T / sqrt(D), axis=-1)
    final = (beta @ v) @ Ww.T + bw            # [B, N]

Algebraic restructuring (exact, modulo fp reassociation):
  *  q.k = x A x^T + r[n] + c[m] + const, with A = Wq^T Wk,
     r[n] = x[n].(Wq^T bk) (row-constant -> drops out of softmax),
     c[m] = x[m].(Wk^T bq) (key-side constant, kept). c is LINEAR in
     x[m], so it folds into the TT operand: c[m] = sum_e g[e] x[m,e]
     with g = Wk^T bq -> add g to every column of TT (see below).
  *  (beta @ v) @ Ww^T = beta @ (x @ (Wv^T Ww^T) + bv.Ww)
     -> the whole V projection collapses into a per-key scalar wv[m].
  *  final[n] = sum_m exp(s[n,m]) wv[m] / sum_m exp(s[n,m]) + bw
     (softmax max-subtraction skipped: logits are O(1) here, exp is safe
      in fp32 — both sums are formed unnormalized and divided at the end).

Sharding: 8 cores = 4 batches x 2 query-halves. Each core computes, for
its 1024 queries n and all 2048 keys m of its batch (keys ordered
local-half-first so all 8 cores run an identical SPMD program):
    TT[e, n]  = sum_f A[f, e] xT[f, n] + g[e]     (phase 1 + DVE add)
    S[n, m]   = sum_e TT[e, n] xT[e, m]           (phase 2, QUERIES on
                                                   partitions, keys free)
    pt        = Exp(S/32/32)                      (ScalarE; accum_out
                                                   gives den = sum_m pt)
    num[n]    = sum_m pt[n, m] wv[m]              (DVE tensor_tensor_reduce
                                                   vs wv replicated row)
Host divides num/den and adds bw.

The orientation (queries on PSUM partitions) makes both softmax sums
FREE-dim reductions, so no PE matmuls are spent on them — the old
key-partition layout burned ~20% of PE time on [128,2]-stationary
reduction matmuls at 0.8% PE efficiency.

All heavy matmuls run in fp8e4 (e4m3) with MatmulPerfMode.DoubleRow:
each instruction contracts TWO 128-deep tiles (stationary [128,2,128],
moving [128,2,512]) at ~1 column/cycle — the 157 TF/s fp8 peak. A is
pre-scaled by 32 on the host so its entries (std ~0.01) sit in e4m3's
normal range; the exp scale compensates (1/1024). Measured numerics:
~1.4e-2 max rel err vs the 2e-2 gate (dominated by fp8 quantization of
x and TT; pt/wv stay bf16, accumulation fp32).

Inputs are host-prearranged into partition-contiguous SBUF images so
the whole input set streams in 8 large DMAs (the per-slice DMA fan-out
of the v1 kernel serialized ~35 descriptor pushes at ~600ns each on the
Sync engine and starved phase 1).
"""

import numpy as np
from contextlib import ExitStack

import ml_dtypes

import concourse.tile as tile
from concourse import bacc, mybir
from concourse.bass_utils import run_bass_kernel_spmd

B = 4
N = 2048
D = 1024
P = 128
ET = D // P          # 8 contraction tiles over D
NQ = N // 2          # 1024 local queries per core
NT = NQ // P         # 8 query tiles
CHUNK = 512          # PSUM bank limit (512 fp32 outputs)
KC = N // CHUNK      # 4 key chunks
NCH = NQ // CHUNK    # 2 query chunks (phase 1 moving)
NP = ET // 2         # 4 DoubleRow contraction pairs
E0T = 3              # e-tiles in phase-1 block 0 (A split a0/a1)
SCALE = 0.03125      # 1/sqrt(D), exact
A_SCALE = 32.0       # host multiplies A,g by this; exp scale divides
WARMUP_MM = 4        # dummy matmuls to lift the PE HAM clock-gate early
F32 = mybir.dt.float32
BF16 = mybir.dt.bfloat16
FP8 = mybir.dt.float8e4
EXP = mybir.ActivationFunctionType.Exp
IDN = mybir.ActivationFunctionType.Identity
DR = mybir.MatmulPerfMode.DoubleRow
ADD = mybir.AluOpType.add
MULT = mybir.AluOpType.mult
AXX = mybir.AxisListType.X

_CACHE = {}


def _build():
    nc = bacc.Bacc(
        "TRN2",
        target_bir_lowering=False,
        debug=False,
        enable_asserts=False,
        num_devices=8,
    )
    # Host-prearranged, partition-contiguous inputs (see _prep):
    #   xq [p, f, n] = x[query n, f*128+p]     (local query half)
    #   xk [p, f, m] = x[key m, f*128+p]       (other half, m >= NQ)
    #   a0 [p, f, e] = 32*A[f*128+p, e]        e in [0, 384)
    #   a1 [p, f, e] = 32*A[f*128+p, 384+e]    e in [0, 640)
    #   gt [p, et]   = 32*g[et*128+p]
    #   wv [p, m]    = wv[m]  (replicated rows, for the DVE reduce)
    # xq/xk are separate tensors so phase 1's critical DMA bytes are
    # only a0+xq (1.4MB) — the key half streams during phase-1 compute.
    xq_d = nc.dram_tensor("xq", [P, ET, NQ], FP8, kind="ExternalInput")
    xk_d = nc.dram_tensor("xk", [P, ET, NQ], FP8, kind="ExternalInput")
    a0_d = nc.dram_tensor("a0", [P, ET, E0T * P], FP8, kind="ExternalInput")
    a1_d = nc.dram_tensor("a1", [P, ET, (ET - E0T) * P], FP8,
                          kind="ExternalInput")
    gt_d = nc.dram_tensor("gt", [P, ET], F32, kind="ExternalInput")
    wv_d = nc.dram_tensor("wv", [P, N], BF16, kind="ExternalInput")
    out_d = nc.dram_tensor("out", [P, NT, 2], F32, kind="ExternalOutput")

    with tile.TileContext(nc) as tc, ExitStack() as ctx:
        const = ctx.enter_context(tc.tile_pool(name="const", bufs=1))
        ptp = ctx.enter_context(tc.tile_pool(name="pt", bufs=4))
        scrp = ctx.enter_context(tc.tile_pool(name="scr", bufs=2))
        workp = ctx.enter_context(
            tc.tile_pool(name="psum_work", bufs=6, space="PSUM")
        )
        wup = ctx.enter_context(
            tc.tile_pool(name="psum_wu", bufs=1, space="PSUM")
        )

        x_sb = const.tile([P, ET, N], FP8)      # xT, queries 0:NQ / keys all
        a0_sb = const.tile([P, ET, E0T * P], FP8)
        a1_sb = const.tile([P, ET, (ET - E0T) * P], FP8)
        tt_sb = const.tile([P, ET, NQ], FP8)    # TT' = 32(A^T xq^T + g)
        gt_sb = const.tile([P, ET], F32)
        wv_sb = const.tile([P, N], BF16)
        den_sb = const.tile([P, NT, KC], F32)   # per-chunk denominators
        num_sb = const.tile([P, NT, KC], F32)   # per-chunk numerators
        out_sb = const.tile([P, NT, 2], F32)
        wu_sb = const.tile([P, CHUNK], BF16)    # warmup operand (garbage ok)
        wu_sink = const.tile([P, 1], F32)

        # PE warm-up: keep TensorE busy until the first (a0, xq) pair
        # lands (~1.5us). No per-step fillers beyond that — queued
        # warmups were measured DELAYING the first real matmul by ~3us
        # (in-order PE), and block 0 is compute-bound after pair 0.
        nc.gpsimd.memset(wu_sb[:], 0.0)
        wu_ps = wup.tile([P, CHUNK], F32)
        wpf = 0
        n_wu = WARMUP_MM + wpf * NP
        wu_iter = iter(range(n_wu))

        def warm(k):
            for _ in range(k):
                w = next(wu_iter, None)
                if w is None:
                    return
                nc.tensor.matmul(
                    wu_ps[:],
                    wu_sb[:, :P],
                    wu_sb[:],
                    start=(w == 0),
                    stop=(w == n_wu - 1),
                )

        warm(WARMUP_MM)

        # Input DMAs. Single-queue transfer bandwidth is only ~60-100GB/s,
        # so the phase-1 critical tensors are split per contraction pair
        # and interleaved (a0 pair i + xq pair i) to fan across queues —
        # the first real matmul needs only (a0 pair 0, xq pair 0), which
        # lands ~1us after issue. Key halves + a1 feed blocks 1-2 /
        # phase 2 (several us later); wv last (first numerator reduce is
        # ~20us in).
        for i in range(NP):
            nc.sync.dma_start(a0_sb[:, 2 * i:2 * i + 2, :],
                              a0_d[:, 2 * i:2 * i + 2, :])
            nc.sync.dma_start(x_sb[:, 2 * i:2 * i + 2, :NQ],
                              xq_d[:, 2 * i:2 * i + 2, :])
        nc.sync.dma_start(gt_sb[:], gt_d[:])
        nc.sync.dma_start(a1_sb[:, :4, :], a1_d[:, :4, :])
        nc.sync.dma_start(a1_sb[:, 4:, :], a1_d[:, 4:, :])
        for i in range(NP):
            nc.sync.dma_start(x_sb[:, 2 * i:2 * i + 2, NQ:],
                              xk_d[:, 2 * i:2 * i + 2, :])
        nc.sync.dma_start(wv_sb[:], wv_d[:])

        def a_slc(ci, e):
            if e < E0T:
                return a0_sb[:, 2 * ci:2 * ci + 2, e * P:(e + 1) * P]
            eo = e - E0T
            return a1_sb[:, 2 * ci:2 * ci + 2, eo * P:(eo + 1) * P]

        # Phase 1: TT[e, n] = sum_f A[f, e-cols]^T . xT[f, n] over the
        # local-query columns. e-blocks of 3 keep 6 PSUM groups open so
        # each arriving (A, x) pair DMA feeds 6 matmuls.
        BLOCKS = [(0, 3), (3, 3), (6, 2)]
        for eb, (e0, blk) in enumerate(BLOCKS):
            pss = [
                [workp.tile([P, CHUNK], F32, name=f"tt_ps_{eb}_{el}_{j}",
                            tag="ps") for j in range(NCH)]
                for el in range(blk)
            ]
            for ci in range(NP):
                for el in range(blk):
                    e = e0 + el
                    for j in range(NCH):
                        nc.tensor.matmul(
                            pss[el][j][:],
                            a_slc(ci, e),
                            x_sb[:, 2 * ci:2 * ci + 2,
                                 j * CHUNK:(j + 1) * CHUNK],
                            start=(ci == 0),
                            stop=(ci == NP - 1),
                            perf_mode=DR,
                        )
                if eb == 0:
                    # absorb the DMA-arrival pacing of the first block
                    warm(wpf)
            for el in range(blk):
                e = e0 + el
                for j in range(NCH):
                    # fused add of the key-side constant g + fp8 cast.
                    # Stays on DVE: counterintuitively, moving these to
                    # the otherwise-idle ScalarE measured SLOWER overall —
                    # the early cast ops keep the DVE queue primed and
                    # its back-to-back pipelining helps it drain the
                    # phase-2 numerator backlog faster.
                    nc.vector.tensor_scalar(
                        tt_sb[:, e, j * CHUNK:(j + 1) * CHUNK],
                        pss[el][j][:],
                        gt_sb[:, e:e + 1],
                        None,
                        ADD,
                    )
            if eb == 0:
                warm(100)  # flush any leftover warmups
                nc.vector.tensor_copy(wu_sink[:], wu_ps[:, :1])

        # Phase 2: per (query-tile nt, key-chunk ch): S accumulates in
        # PSUM, exp with free-dim accumulation (denominator), DVE
        # multiply + chunk reduce (numerator partial). Per-chunk DVE ops
        # pay more fixed overhead than full-width per-nt passes, but the
        # measured end-to-end is FASTER this way — wider passes bunch at
        # tile boundaries and the last tile's work queues behind them.
        # (tensor_tensor_reduce would fuse the two DVE ops, but that
        # opcode hard-faults this runtime's exec unit.)
        for nt in range(NT):
            for ch in range(KC):
                ps = workp.tile([P, CHUNK], F32, name=f"s_ps_{nt}_{ch}",
                                tag="ps")
                for ci in range(NP):
                    nc.tensor.matmul(
                        ps[:],
                        tt_sb[:, 2 * ci:2 * ci + 2, nt * P:(nt + 1) * P],
                        x_sb[:, 2 * ci:2 * ci + 2,
                             ch * CHUNK:(ch + 1) * CHUNK],
                        start=(ci == 0),
                        stop=(ci == NP - 1),
                        perf_mode=DR,
                    )
                pt = ptp.tile([P, CHUNK], BF16, name=f"pt_{nt}_{ch}",
                              tag="pt")
                nc.scalar.activation(
                    pt[:], ps[:], EXP, scale=SCALE / A_SCALE,
                    accum_out=den_sb[:, nt, ch:ch + 1],
                )
                scr = scrp.tile([P, CHUNK], BF16, name=f"scr_{nt}_{ch}",
                                tag="scr")
                nc.vector.tensor_tensor(
                    scr[:], pt[:], wv_sb[:, ch * CHUNK:(ch + 1) * CHUNK],
                    MULT,
                )
                nc.vector.tensor_reduce(
                    num_sb[:, nt, ch:ch + 1], scr[:], AXX, ADD,
                )

        # Phase 3: fold the KC per-chunk partials and ship (num, den).
        nc.vector.tensor_reduce(out_sb[:, :, 0:1], num_sb[:], AXX, ADD)
        nc.vector.tensor_reduce(out_sb[:, :, 1:2], den_sb[:], AXX, ADD)
        nc.sync.dma_start(out_d[:], out_sb[:])

    nc.compile()
    return nc


def _get_nc():
    if "nc" not in _CACHE:
        _CACHE["nc"] = _build()
    return _CACHE["nc"]


def _prep(x, Wq, bq, Wk, bk, Wv, bv, Ww, bw):
    """Host-side sharding + weight folding -> per-core input maps."""
    x = np.asarray(x, dtype=np.float32)
    Wq = np.asarray(Wq, dtype=np.float32)
    bq = np.asarray(bq, dtype=np.float32)
    Wk = np.asarray(Wk, dtype=np.float32)
    bk = np.asarray(bk, dtype=np.float32)
    Wv = np.asarray(Wv, dtype=np.float32)
    bv = np.asarray(bv, dtype=np.float32)
    Ww = np.asarray(Ww, dtype=np.float32)
    f8 = ml_dtypes.float8_e4m3
    bf = ml_dtypes.bfloat16

    # Host-side weight folding (cheap: one 1024^3 sgemm + matvecs).
    A = (Wq.T @ Wk) * A_SCALE                       # [f, e], fp8-range
    g = (Wk.T @ bq) * A_SCALE                       # key-side logit constant
    wv_eff = Wv.T @ Ww[0]                           # collapsed V @ Ww^T
    cvw = float(bv @ Ww[0])
    wv_all = x @ wv_eff + cvw                       # [B, N]

    a_im = np.ascontiguousarray(
        A.reshape(ET, P, D).transpose(1, 0, 2))     # [p, f, e]
    a0 = np.ascontiguousarray(a_im[:, :, :E0T * P]).astype(f8)
    a1 = np.ascontiguousarray(a_im[:, :, E0T * P:]).astype(f8)
    gt = np.ascontiguousarray(g.reshape(ET, P).T)   # [p, et] f32

    in_maps = []
    for core in range(8):
        b, h = divmod(core, 2)
        lo = np.arange(h * NQ, (h + 1) * NQ)
        hi = np.arange((1 - h) * NQ, (2 - h) * NQ)
        order = np.concatenate([lo, hi])            # keys: local half first
        xqim = np.ascontiguousarray(
            x[b][lo].T.reshape(ET, P, NQ).transpose(1, 0, 2)
        ).astype(f8)                                # [p, f, n]
        xkim = np.ascontiguousarray(
            x[b][hi].T.reshape(ET, P, NQ).transpose(1, 0, 2)
        ).astype(f8)                                # [p, f, m-NQ]
        wvr = np.ascontiguousarray(
            np.broadcast_to(wv_all[b][order][None, :].astype(bf), (P, N))
        )
        in_maps.append({"xq": xqim, "xk": xkim, "a0": a0, "a1": a1,
                        "gt": gt, "wv": wvr})
    return in_maps


def _gather(res, bw):
    bw = np.asarray(bw, dtype=np.float32)
    final = np.empty((B, N), dtype=np.float32)
    for core in range(8):
        b, h = divmod(core, 2)
        o = res.results[core]["out"]                # [p, nt, 2]
        num = o[:, :, 0].T.reshape(NQ)              # n = nt*128 + p
        den = o[:, :, 1].T.reshape(NQ)
        final[b, h * NQ:(h + 1) * NQ] = num / den + bw[0]
    return final


def kernel(x, Wq, bq, Wk, bk, Wv, bv, Ww, bw):
    nc = _get_nc()
    in_maps = _prep(x, Wq, bq, Wk, bk, Wv, bv, Ww, bw)
    res = run_bass_kernel_spmd(nc, in_maps, core_ids=list(range(8)))
    return _gather(res, bw)


def run_profiled(inputs, trace_cores=(0,)):
    """Run once with NTFF profiling; returns BassKernelResults."""
    nc = _get_nc()
    in_maps = _prep(**inputs)
    res = run_bass_kernel_spmd(
        nc, in_maps, core_ids=list(range(8)), trace=True,
        trace_cores=list(trace_cores),
    )
    return res


# revision 33
# speedup vs baseline: 1.0331x; 1.0208x over previous
"""Trainium2 Bass kernel for nn_CAAN_78323023610440.

Reference computation (per batch b):
    q = x @ Wq.T + bq;  k = x @ Wk.T + bk;  v = x @ Wv.T + bv
    beta = softmax(q @ k.T / sqrt(D), axis=-1)
    final = (beta @ v) @ Ww.T + bw            # [B, N]

Algebraic restructuring (exact, modulo fp reassociation):
  *  q.k = x A x^T + r[n] + c[m] + const, with A = Wq^T Wk,
     r[n] = x[n].(Wq^T bk) (row-constant -> drops out of softmax),
     c[m] = x[m].(Wk^T bq) (key-side constant, kept). c is LINEAR in
     x[m], so it folds into the TT operand: c[m] = sum_e g[e] x[m,e]
     with g = Wk^T bq -> add g to every column of TT (see below).
  *  (beta @ v) @ Ww^T = beta @ (x @ (Wv^T Ww^T) + bv.Ww)
     -> the whole V projection collapses into a per-key scalar wv[m].
  *  final[n] = sum_m exp(s[n,m]) wv[m] / sum_m exp(s[n,m]) + bw
     (softmax max-subtraction skipped: logits are O(1) here, exp is safe
      in fp32 — both sums are formed unnormalized and divided at the end).

Sharding: 8 cores = 4 batches x 2 query-halves. Each core computes, for
its 1024 queries n and all 2048 keys m of its batch (keys ordered
local-half-first so all 8 cores run an identical SPMD program):
    TT[e, n]  = sum_f A[f, e] xT[f, n] + g[e]     (phase 1 + DVE add)
    S[n, m]   = sum_e TT[e, n] xT[e, m]           (phase 2, QUERIES on
                                                   partitions, keys free)
    pt        = Exp(S/32/32)                      (ScalarE; accum_out
                                                   gives den = sum_m pt)
    num[n]    = sum_m pt[n, m] wv[m]              (DVE tensor_tensor_reduce
                                                   vs wv replicated row)
Host divides num/den and adds bw.

The orientation (queries on PSUM partitions) makes both softmax sums
FREE-dim reductions, so no PE matmuls are spent on them — the old
key-partition layout burned ~20% of PE time on [128,2]-stationary
reduction matmuls at 0.8% PE efficiency.

All heavy matmuls run in fp8e4 (e4m3) with MatmulPerfMode.DoubleRow:
each instruction contracts TWO 128-deep tiles (stationary [128,2,128],
moving [128,2,512]) at ~1 column/cycle — the 157 TF/s fp8 peak. A is
pre-scaled by 32 on the host so its entries (std ~0.01) sit in e4m3's
normal range; the exp scale compensates (1/1024). Measured numerics:
~1.4e-2 max rel err vs the 2e-2 gate (dominated by fp8 quantization of
x and TT; pt/wv stay bf16, accumulation fp32).

Inputs are host-prearranged into partition-contiguous SBUF images so
the whole input set streams in 8 large DMAs (the per-slice DMA fan-out
of the v1 kernel serialized ~35 descriptor pushes at ~600ns each on the
Sync engine and starved phase 1).
"""

import numpy as np
from contextlib import ExitStack

import ml_dtypes

import concourse.tile as tile
from concourse import bacc, mybir
from concourse.bass_utils import run_bass_kernel_spmd

B = 4
N = 2048
D = 1024
P = 128
ET = D // P          # 8 contraction tiles over D
NQ = N // 2          # 1024 local queries per core
NT = NQ // P         # 8 query tiles
CHUNK = 512          # PSUM bank limit (512 fp32 outputs)
KC = N // CHUNK      # 4 key chunks
NCH = NQ // CHUNK    # 2 query chunks (phase 1 moving)
NP = ET // 2         # 4 DoubleRow contraction pairs
E0T = 3              # e-tiles in phase-1 block 0 (A split a0/a1)
SCALE = 0.03125      # 1/sqrt(D), exact
A_SCALE = 32.0       # host multiplies A,g by this; exp scale divides
WARMUP_MM = 4        # dummy matmuls to lift the PE HAM clock-gate early
F32 = mybir.dt.float32
BF16 = mybir.dt.bfloat16
FP8 = mybir.dt.float8e4
EXP = mybir.ActivationFunctionType.Exp
IDN = mybir.ActivationFunctionType.Identity
DR = mybir.MatmulPerfMode.DoubleRow
ADD = mybir.AluOpType.add
MULT = mybir.AluOpType.mult
AXX = mybir.AxisListType.X

_CACHE = {}


def _build():
    nc = bacc.Bacc(
        "TRN2",
        target_bir_lowering=False,
        debug=False,
        enable_asserts=False,
        num_devices=8,
    )
    # Host-prearranged, partition-contiguous inputs (see _prep):
    #   xq [p, f, n] = x[query n, f*128+p]     (local query half)
    #   xk [p, f, m] = x[key m, f*128+p]       (other half, m >= NQ)
    #   a0 [p, f, e] = 32*A[f*128+p, e]        e in [0, 384)
    #   a1 [p, f, e] = 32*A[f*128+p, 384+e]    e in [0, 640)
    #   gt [p, et]   = 32*g[et*128+p]
    #   wv [p, m]    = wv[m]  (replicated rows, for the DVE reduce)
    # xq/xk are separate tensors so phase 1's critical DMA bytes are
    # only a0+xq (1.4MB) — the key half streams during phase-1 compute.
    xq_d = nc.dram_tensor("xq", [P, ET, NQ], FP8, kind="ExternalInput")
    xk_d = nc.dram_tensor("xk", [P, ET, NQ], FP8, kind="ExternalInput")
    a0_d = nc.dram_tensor("a0", [P, ET, E0T * P], FP8, kind="ExternalInput")
    a1_d = nc.dram_tensor("a1", [P, ET, (ET - E0T) * P], FP8,
                          kind="ExternalInput")
    gt_d = nc.dram_tensor("gt", [P, ET], F32, kind="ExternalInput")
    wv_d = nc.dram_tensor("wv", [P, N], BF16, kind="ExternalInput")
    out_d = nc.dram_tensor("out", [P, NT, 2], F32, kind="ExternalOutput")

    with tile.TileContext(nc) as tc, ExitStack() as ctx:
        const = ctx.enter_context(tc.tile_pool(name="const", bufs=1))
        ptp = ctx.enter_context(tc.tile_pool(name="pt", bufs=4))
        scrp = ctx.enter_context(tc.tile_pool(name="scr", bufs=2))
        workp = ctx.enter_context(
            tc.tile_pool(name="psum_work", bufs=6, space="PSUM")
        )
        wup = ctx.enter_context(
            tc.tile_pool(name="psum_wu", bufs=1, space="PSUM")
        )

        x_sb = const.tile([P, ET, N], FP8)      # xT, queries 0:NQ / keys all
        a0_sb = const.tile([P, ET, E0T * P], FP8)
        a1_sb = const.tile([P, ET, (ET - E0T) * P], FP8)
        tt_sb = const.tile([P, ET, NQ], FP8)    # TT' = 32(A^T xq^T + g)
        gt_sb = const.tile([P, ET], F32)
        wv_sb = const.tile([P, N], BF16)
        den_sb = const.tile([P, NT, KC], F32)   # per-chunk denominators
        num_sb = const.tile([P, NT, KC], F32)   # per-chunk numerators
        out_sb = const.tile([P, NT, 2], F32)
        wu_sb = const.tile([P, CHUNK], BF16)    # warmup operand (garbage ok)
        wu_sink = const.tile([P, 1], F32)

        # PE warm-up: keep TensorE busy until the first (a0, xq) pair
        # lands (~1.5us). No per-step fillers beyond that — queued
        # warmups were measured DELAYING the first real matmul by ~3us
        # (in-order PE), and block 0 is compute-bound after pair 0.
        nc.gpsimd.memset(wu_sb[:], 0.0)
        wu_ps = wup.tile([P, CHUNK], F32)
        wpf = 0
        n_wu = WARMUP_MM + wpf * NP
        wu_iter = iter(range(n_wu))

        def warm(k):
            for _ in range(k):
                w = next(wu_iter, None)
                if w is None:
                    return
                nc.tensor.matmul(
                    wu_ps[:],
                    wu_sb[:, :P],
                    wu_sb[:],
                    start=(w == 0),
                    stop=(w == n_wu - 1),
                )

        warm(WARMUP_MM)

        # Input DMAs. Single-queue transfer bandwidth is only ~60-100GB/s,
        # so the phase-1 critical tensors are split per contraction pair
        # and interleaved (a0 pair i + xq pair i) to fan across queues —
        # the first real matmul needs only (a0 pair 0, xq pair 0), which
        # lands ~1us after issue. Key halves + a1 feed blocks 1-2 /
        # phase 2 (several us later); wv last (first numerator reduce is
        # ~20us in).
        for i in range(NP):
            nc.sync.dma_start(a0_sb[:, 2 * i:2 * i + 2, :],
                              a0_d[:, 2 * i:2 * i + 2, :])
            for j in range(NCH):
                nc.sync.dma_start(
                    x_sb[:, 2 * i:2 * i + 2, j * CHUNK:(j + 1) * CHUNK],
                    xq_d[:, 2 * i:2 * i + 2, j * CHUNK:(j + 1) * CHUNK])
        nc.sync.dma_start(gt_sb[:], gt_d[:])
        for i in range(NP):
            nc.sync.dma_start(a1_sb[:, 2 * i:2 * i + 2, :],
                              a1_d[:, 2 * i:2 * i + 2, :])
        for i in range(NP):
            nc.sync.dma_start(x_sb[:, 2 * i:2 * i + 2, NQ:],
                              xk_d[:, 2 * i:2 * i + 2, :])
        nc.sync.dma_start(wv_sb[:], wv_d[:])

        def a_slc(ci, e):
            if e < E0T:
                return a0_sb[:, 2 * ci:2 * ci + 2, e * P:(e + 1) * P]
            eo = e - E0T
            return a1_sb[:, 2 * ci:2 * ci + 2, eo * P:(eo + 1) * P]

        # Phase 1: TT[e, n] = sum_f A[f, e-cols]^T . xT[f, n] over the
        # local-query columns. e-blocks of 3 keep 6 PSUM groups open so
        # each arriving (A, x) pair DMA feeds 6 matmuls.
        BLOCKS = [(0, 3), (3, 3), (6, 2)]
        for eb, (e0, blk) in enumerate(BLOCKS):
            pss = [
                [workp.tile([P, CHUNK], F32, name=f"tt_ps_{eb}_{el}_{j}",
                            tag="ps") for j in range(NCH)]
                for el in range(blk)
            ]
            for ci in range(NP):
                for el in range(blk):
                    e = e0 + el
                    for j in range(NCH):
                        nc.tensor.matmul(
                            pss[el][j][:],
                            a_slc(ci, e),
                            x_sb[:, 2 * ci:2 * ci + 2,
                                 j * CHUNK:(j + 1) * CHUNK],
                            start=(ci == 0),
                            stop=(ci == NP - 1),
                            perf_mode=DR,
                        )
                if eb == 0:
                    # absorb the DMA-arrival pacing of the first block
                    warm(wpf)
            for el in range(blk):
                e = e0 + el
                for j in range(NCH):
                    # fused add of the key-side constant g + fp8 cast.
                    # Stays on DVE: counterintuitively, moving these to
                    # the otherwise-idle ScalarE measured SLOWER overall —
                    # the early cast ops keep the DVE queue primed and
                    # its back-to-back pipelining helps it drain the
                    # phase-2 numerator backlog faster.
                    nc.vector.tensor_scalar(
                        tt_sb[:, e, j * CHUNK:(j + 1) * CHUNK],
                        pss[el][j][:],
                        gt_sb[:, e:e + 1],
                        None,
                        ADD,
                    )
            if eb == 0:
                warm(100)  # flush any leftover warmups
                nc.vector.tensor_copy(wu_sink[:], wu_ps[:, :1])

        # Phase 2: per (query-tile nt, key-chunk ch): S accumulates in
        # PSUM, exp with free-dim accumulation (denominator), DVE
        # multiply + chunk reduce (numerator partial). Per-chunk DVE ops
        # pay more fixed overhead than full-width per-nt passes, but the
        # measured end-to-end is FASTER this way — wider passes bunch at
        # tile boundaries and the last tile's work queues behind them.
        # (tensor_tensor_reduce would fuse the two DVE ops, but that
        # opcode hard-faults this runtime's exec unit.)
        for nt in range(NT):
            for ch in range(KC):
                ps = workp.tile([P, CHUNK], F32, name=f"s_ps_{nt}_{ch}",
                                tag="ps")
                for ci in range(NP):
                    nc.tensor.matmul(
                        ps[:],
                        tt_sb[:, 2 * ci:2 * ci + 2, nt * P:(nt + 1) * P],
                        x_sb[:, 2 * ci:2 * ci + 2,
                             ch * CHUNK:(ch + 1) * CHUNK],
                        start=(ci == 0),
                        stop=(ci == NP - 1),
                        perf_mode=DR,
                    )
                pt = ptp.tile([P, CHUNK], BF16, name=f"pt_{nt}_{ch}",
                              tag="pt")
                nc.scalar.activation(
                    pt[:], ps[:], EXP, scale=SCALE / A_SCALE,
                    accum_out=den_sb[:, nt, ch:ch + 1],
                )
                scr = scrp.tile([P, CHUNK], BF16, name=f"scr_{nt}_{ch}",
                                tag="scr")
                nc.vector.tensor_tensor(
                    scr[:], pt[:], wv_sb[:, ch * CHUNK:(ch + 1) * CHUNK],
                    MULT,
                )
                nc.vector.tensor_reduce(
                    num_sb[:, nt, ch:ch + 1], scr[:], AXX, ADD,
                )

        # Phase 3: fold the KC per-chunk partials and ship (num, den).
        nc.vector.tensor_reduce(out_sb[:, :, 0:1], num_sb[:], AXX, ADD)
        nc.vector.tensor_reduce(out_sb[:, :, 1:2], den_sb[:], AXX, ADD)
        nc.sync.dma_start(out_d[:], out_sb[:])

    nc.compile()
    return nc


def _get_nc():
    if "nc" not in _CACHE:
        _CACHE["nc"] = _build()
    return _CACHE["nc"]


def _prep(x, Wq, bq, Wk, bk, Wv, bv, Ww, bw):
    """Host-side sharding + weight folding -> per-core input maps."""
    x = np.asarray(x, dtype=np.float32)
    Wq = np.asarray(Wq, dtype=np.float32)
    bq = np.asarray(bq, dtype=np.float32)
    Wk = np.asarray(Wk, dtype=np.float32)
    bk = np.asarray(bk, dtype=np.float32)
    Wv = np.asarray(Wv, dtype=np.float32)
    bv = np.asarray(bv, dtype=np.float32)
    Ww = np.asarray(Ww, dtype=np.float32)
    f8 = ml_dtypes.float8_e4m3
    bf = ml_dtypes.bfloat16

    # Host-side weight folding (cheap: one 1024^3 sgemm + matvecs).
    A = (Wq.T @ Wk) * A_SCALE                       # [f, e], fp8-range
    g = (Wk.T @ bq) * A_SCALE                       # key-side logit constant
    wv_eff = Wv.T @ Ww[0]                           # collapsed V @ Ww^T
    cvw = float(bv @ Ww[0])
    wv_all = x @ wv_eff + cvw                       # [B, N]

    a_im = np.ascontiguousarray(
        A.reshape(ET, P, D).transpose(1, 0, 2))     # [p, f, e]
    a0 = np.ascontiguousarray(a_im[:, :, :E0T * P]).astype(f8)
    a1 = np.ascontiguousarray(a_im[:, :, E0T * P:]).astype(f8)
    gt = np.ascontiguousarray(g.reshape(ET, P).T)   # [p, et] f32

    in_maps = []
    for core in range(8):
        b, h = divmod(core, 2)
        lo = np.arange(h * NQ, (h + 1) * NQ)
        hi = np.arange((1 - h) * NQ, (2 - h) * NQ)
        order = np.concatenate([lo, hi])            # keys: local half first
        xqim = np.ascontiguousarray(
            x[b][lo].T.reshape(ET, P, NQ).transpose(1, 0, 2)
        ).astype(f8)                                # [p, f, n]
        xkim = np.ascontiguousarray(
            x[b][hi].T.reshape(ET, P, NQ).transpose(1, 0, 2)
        ).astype(f8)                                # [p, f, m-NQ]
        wvr = np.ascontiguousarray(
            np.broadcast_to(wv_all[b][order][None, :].astype(bf), (P, N))
        )
        in_maps.append({"xq": xqim, "xk": xkim, "a0": a0, "a1": a1,
                        "gt": gt, "wv": wvr})
    return in_maps


def _gather(res, bw):
    bw = np.asarray(bw, dtype=np.float32)
    final = np.empty((B, N), dtype=np.float32)
    for core in range(8):
        b, h = divmod(core, 2)
        o = res.results[core]["out"]                # [p, nt, 2]
        num = o[:, :, 0].T.reshape(NQ)              # n = nt*128 + p
        den = o[:, :, 1].T.reshape(NQ)
        final[b, h * NQ:(h + 1) * NQ] = num / den + bw[0]
    return final


def kernel(x, Wq, bq, Wk, bk, Wv, bv, Ww, bw):
    nc = _get_nc()
    in_maps = _prep(x, Wq, bq, Wk, bk, Wv, bv, Ww, bw)
    res = run_bass_kernel_spmd(nc, in_maps, core_ids=list(range(8)))
    return _gather(res, bw)


def run_profiled(inputs, trace_cores=(0,)):
    """Run once with NTFF profiling; returns BassKernelResults."""
    nc = _get_nc()
    in_maps = _prep(**inputs)
    res = run_bass_kernel_spmd(
        nc, in_maps, core_ids=list(range(8)), trace=True,
        trace_cores=list(trace_cores),
    )
    return res
